# revision 38
# baseline (speedup 1.0000x reference)
"""Trainium2 Bass kernel for AttnBlock (GroupNorm + 1x1-conv QKV self-attention
+ output proj + residual) on x: [4, 512, 64, 64] fp32, distributed over 8
NeuronCores.

Sharding: data-parallel over batch (4) x sequence-parallel over the N=H*W=4096
token axis (2 halves) = 8 cores. Each core receives the full image of its
batch element with the token axis rotated so that its 2048 query tokens come
first; it computes GroupNorm + K/V for all 4096 tokens (duplicated within the
batch pair -- no collectives needed) and Q/attention/output only for its 2048
queries. The host gathers the 8 [512, 2048] outputs back into [4, 512, 64, 64].

All matmuls (QKV projections, scores, attention@V, O-projection) run in
fp8e4m3 with MatmulPerfMode.DoubleRow (2 fp8 weights per PE cell -> 2x
MACs/cycle, ~228ns per [256x128x512] instruction = ~94% of fp8 peak), with
fp32 PSUM accumulation. Key structure:
- x ships in fp8; GroupNorm stats are computed from the fp8 tiles (the
  ~0.1% var bias is negligible) and folded into the projections:
  wk@(s*x+t) = (wk*s)@x + (wk@t). The scaled weights land in fp8 DoubleRow
  layout [128, 2, C].
- The K-projection needs no bias at all: q_i.(k_j + c) with c constant over
  j is softmax-invariant, so bk and the wk@t correction drop. Projection
  order K, V, Q so only the last (Q) waits on its bias columns.
- Scores are computed transposed (S^T = K^T Q per key tile) so softmax and
  the attention@V contraction need no transposes. exp runs on ACT straight
  out of PSUM with bias -2 (keeps exp <= ~70, under fp8e4's 240 max) and
  writes fp8 into the DoubleRow-paired p tiles.
- Softmax denominator partials accumulate on DVE+GPSIMD off the critical
  path; the partition reduce runs transposed (4x [128,1] matmuls) so the
  DVE reciprocal is 128-wide (~0.1us, not a 4us single-lane [1,512] op),
  then a PE transpose + 4 selector matmuls broadcast 1/den to all
  partitions. The normalized attention output drains straight to fp8
  DoubleRow pairs feeding the fp8 O-projection.
- A 6-pair score/exp lookahead across query blocks keeps the PE dense
  through each block's normalize/O-proj/store tail.
- The residual is added from a host-precomputed fp32 x+bo tensor; output
  stores spread across the sync/ACT/GPSIMD DMA queues.
Measured: ~235us HW exec on 8 cores (baseline bf16 version: ~378us),
rel l2 vs fp32 reference ~4.0e-3.
"""

import numpy as np
import ml_dtypes

B, C, H, W = 4, 512, 64, 64
N = H * W            # 4096 tokens
NQ = N // 2          # 2048 queries per core
P = 128              # partitions
CT = C // P          # 4 channel tiles
CP = CT // 2         # 2 channel-tile pairs (DoubleRow)
JT = N // P          # 32 key/token tiles
JP = JT // 2         # 16 key-tile pairs (DoubleRow)
IBS = 512            # query block (free dim of score matmuls)
IB = NQ // IBS       # 4 query blocks per core
NCH = N // IBS       # 8 n-chunks for full-N projections
GROUPS = 32
GSIZE = C // GROUPS  # 16 channels per group
EPS = 1e-6
SM_SCALE = float(C) ** -0.5
EXP_BIAS = -2.0      # exp(s - 2): max score ~6.2 -> exp <= ~70 < 240 fp8e4 max

N_CORES = 8

_cache = {}


def _build_nc():
    import concourse.bass as bass
    import concourse.mybir as mybir
    import concourse.tile as tile
    from concourse import bacc

    f32 = mybir.dt.float32
    bf16 = mybir.dt.bfloat16
    f8 = mybir.dt.float8e4
    ID = mybir.ActivationFunctionType.Identity
    EXP = mybir.ActivationFunctionType.Exp
    SQRT = mybir.ActivationFunctionType.Sqrt
    DR = mybir.MatmulPerfMode.DoubleRow

    nc = bacc.Bacc("TRN2")

    xr_d = nc.declare_dram_parameter("xr", [C, N], f8, isOutput=False)
    w_d = {
        name: nc.declare_dram_parameter(name, [C, C], bf16, isOutput=False)
        for name in ("wqT", "wkT", "wvT")
    }
    wo8_d = nc.declare_dram_parameter("woT8", [C, C], f8, isOutput=False)
    cols_d = nc.declare_dram_parameter("cols", [C, 6], f32, isOutput=False)
    xqb_d = nc.declare_dram_parameter("xqb", [C, NQ], f32, isOutput=False)
    id_d = nc.declare_dram_parameter("ident", [P, P], f32, isOutput=False)
    ej_d = nc.declare_dram_parameter("ej", [4, 4 * P], f32, isOutput=False)
    inda_d = nc.declare_dram_parameter("ind_a", [P, CT * GROUPS], bf16, isOutput=False)
    indb_d = nc.declare_dram_parameter("ind_b", [GROUPS, CT * P], bf16, isOutput=False)
    out_d = nc.declare_dram_parameter("out", [C, NQ], f32, isOutput=True)

    with tile.TileContext(nc) as tc:
        from contextlib import ExitStack

        with ExitStack() as ctx:
            const = ctx.enter_context(tc.tile_pool(name="const", bufs=1))
            pp_mm = ctx.enter_context(tc.tile_pool(name="pp_mm", bufs=3, space="PSUM"))
            pp_av = ctx.enter_context(tc.tile_pool(name="pp_av", bufs=4, space="PSUM"))
            pp_sm = ctx.enter_context(tc.tile_pool(name="pp_sm", bufs=1, space="PSUM"))

            # ---- batched small constants (few DMAs; issued after x) ----
            cols_t = [const.tile([P, 6], f32, tag=f"cols{t}", name=f"cols{t}")
                      for t in range(CT)]
            inda_t = const.tile([P, CT * GROUPS], bf16, tag="inda", name="inda")
            indb_t = const.tile([GROUPS, CT * P], bf16, tag="indb", name="indb")
            col_sb = {nm: [cols_t[t][:, i:i + 1] for t in range(CT)]
                      for i, nm in enumerate(("bq", "bk", "bv", "bo",
                                              "gamma", "beta"))}
            inda_sb = [inda_t[:, t * GROUPS:(t + 1) * GROUPS] for t in range(CT)]
            indb_sb = [indb_t[:, t * P:(t + 1) * P] for t in range(CT)]

            ones_colf = const.tile([P, 1], f32, tag="ones_colf", name="ones_colf")
            nc.vector.memset(ones_colf, 1.0)
            ones_rowf = const.tile([1, P], f32, tag="ones_rowf", name="ones_rowf")
            nc.vector.memset(ones_rowf, 1.0)
            ebias = const.tile([P, 1], f32, tag="ebias", name="ebias")
            nc.vector.memset(ebias, EXP_BIAS)
            ident_sb = const.tile([P, P], f32, tag="ident", name="ident")
            ej_sb = const.tile([4, 4 * P], f32, tag="ej", name="ej")

            stat_pool = ctx.enter_context(tc.tile_pool(name="stat", bufs=4 * CT))

            k_pool = ctx.enter_context(tc.tile_pool(name="k", bufs=CP))
            v_pool = ctx.enter_context(tc.tile_pool(name="v", bufs=JP))
            q_pool = ctx.enter_context(tc.tile_pool(name="q", bufs=CP))
            # DoubleRow-paired K/Q: [p, i, j] = val[ch = pair*256 + i*128 + p, j]
            k2_sb = [k_pool.tile([P, 2, N], f8, tag="k", name="k")
                     for _ in range(CP)]
            q2_sb = [q_pool.tile([P, 2, NQ], f8, tag="q", name="q")
                     for _ in range(CP)]

            # ---- phase 1: x load (2 HW-DGE queues) + GroupNorm stats ----
            # stats for tiles 0,2,3 via DVE bn_stats; tile 1 via ACT
            # Square/Identity with accum_out (free-dim sums) to halve the
            # serial DVE chain on the critical path.
            mv_sb = []
            with tc.tile_pool(name="xr", bufs=CP) as xr_pool:
                # [p, i, n] = x[ch = pair*256 + i*128 + p, n] in fp8
                x2_sb = [xr_pool.tile([P, 2, N], f8, tag="xr", name="xr")
                         for _ in range(CP)]
                xsl = [x2_sb[t // 2][:, t % 2, :] for t in range(CT)]
                st_sb = []
                acc_cols = []
                # tile 0 lands as 512-wide chunks so the first bn_stats can
                # start ~2us earlier; the rest as 1024-wide as before
                order = [(0, slice(ch * 512, (ch + 1) * 512),
                          nc.sync if ch % 2 == 0 else nc.scalar)
                         for ch in range(4)]
                order += [(t, slice(ch * (N // 4), (ch + 1) * (N // 4)), eng)
                          for t, ch, eng in
                          [(0, 2, nc.sync), (0, 3, nc.scalar),
                           (1, 0, nc.scalar), (1, 1, nc.scalar),
                           (2, 0, nc.sync), (2, 1, nc.sync),
                           (1, 2, nc.scalar), (1, 3, nc.scalar),
                           (2, 2, nc.sync), (2, 3, nc.sync),
                           (3, 0, nc.sync), (3, 2, nc.scalar),
                           (3, 1, nc.sync), (3, 3, nc.scalar)]]
                for t, csl, eng in order:
                    eng.dma_start(out=x2_sb[t // 2][:, t % 2, csl],
                                  in_=xr_d[t * P:(t + 1) * P, csl])

                # HAM warm-up: ~56 junk fp8 matmuls bridge the stats-bound
                # head (PE otherwise idle +10..+25us) so the PE clock gate is
                # already at 8/8 when the real projection stream begins.
                # Sized to finish just before the stats matmuls even if the
                # first ~7us runs cold.
                junk_ps = pp_sm.tile([P, IBS], f32, tag="den", name="junk")
                for _ in range(56):
                    nc.tensor.matmul(junk_ps,
                                     lhsT=x2_sb[0][:, 0, 0:P],
                                     rhs=x2_sb[0][:, 0, 0:IBS],
                                     start=True, stop=True)
                for t in range(CT):
                    xt_g = xsl[t].rearrange("p (s f) -> p s f", f=512)
                    if t != 1:
                        st = stat_pool.tile([P, N // 512, 6], f32, tag="bnst",
                                            name="bnst")
                        sums = None
                        for s in range(N // 512):
                            nc.vector.bn_stats(out=st[:, s, :],
                                               in_=xt_g[:, s, :])
                    else:
                        st = None
                        sums = stat_pool.tile([P, 2, N // 512], f32, tag="acs",
                                              name="acs")
                        for s in range(N // 512):
                            scr = stat_pool.tile([P, 512], bf16, tag="scr",
                                                 name="scr", bufs=2)
                            nc.scalar.activation(
                                out=scr, in_=xt_g[:, s, :],
                                func=mybir.ActivationFunctionType.Square,
                                accum_out=sums[:, 1, s:s + 1])
                            nc.scalar.activation(
                                out=scr, in_=xt_g[:, s, :], func=ID,
                                accum_out=sums[:, 0, s:s + 1])
                    st_sb.append(st)
                    acc_cols.append(sums)

                # batched consts + weights + bv now (queues free after x)
                nc.sync.dma_start(out=inda_t, in_=inda_d[:, :])
                nc.sync.dma_start(out=indb_t, in_=indb_d[:, :])
                nc.sync.dma_start(out=ident_sb, in_=id_d[:, :])
                nc.sync.dma_start(out=ej_sb, in_=ej_d[:, :])
                for t in range(CT):
                    nc.sync.dma_start(out=cols_t[t],
                                      in_=cols_d[t * P:(t + 1) * P, :])
                worig_cm = tc.tile_pool(name="worig", bufs=1)
                worig_pool = worig_cm.__enter__()
                w_sb = {}
                for name in ("wkT", "wqT", "wvT"):
                    tiles = []
                    for t in range(CT):
                        tw = worig_pool.tile([P, C], bf16, tag=f"{name}{t}",
                                             name=f"{name}{t}")
                        nc.sync.dma_start(out=tw,
                                          in_=w_d[name][t * P:(t + 1) * P, :])
                        tiles.append(tw)
                    w_sb[name] = tiles
                # O-projection weights, fp8 DoubleRow layout (host-cast)
                wo2 = [const.tile([P, 2, C], f8, tag=f"wo2{pr}",
                                  name=f"wo2{pr}") for pr in range(CP)]
                for pr in range(CP):
                    for i in range(2):
                        nc.sync.dma_start(
                            out=wo2[pr][:, i, :],
                            in_=wo8_d[(2 * pr + i) * P:(2 * pr + i + 1) * P, :])
                bv_row = const.tile([1, C], f32, tag="bv_row", name="bv_row")
                nc.sync.dma_start(
                    out=bv_row,
                    in_=cols_d[:, 2:3].rearrange("c one -> one c"))

                for t in range(CT):
                    mv = stat_pool.tile([P, 2], f32, tag="mv", name="mv")
                    if st_sb[t] is not None:
                        nc.vector.bn_aggr(out=mv, in_=st_sb[t])
                        # mv = [mean, var] -> [mean, E[x^2]]
                        msq = stat_pool.tile([P, 1], f32, tag="msq", name="msq")
                        nc.vector.tensor_mul(msq, mv[:, 0:1], mv[:, 0:1])
                        nc.vector.tensor_add(mv[:, 1:2], mv[:, 1:2], msq)
                    else:
                        # sums[:, s, 0]=sum(x), [:, s, 1]=sum(x^2) per 512-chunk
                        sred = stat_pool.tile([P, 2], f32, tag="sred", name="sred")
                        nc.vector.tensor_reduce(
                            out=sred, in_=acc_cols[t],
                            op=mybir.AluOpType.add, axis=mybir.AxisListType.X)
                        nc.vector.tensor_scalar_mul(mv, sred, 1.0 / N)
                    mvb = stat_pool.tile([P, 2], bf16, tag="mvb", name="mvb")
                    nc.vector.tensor_copy(out=mvb, in_=mv)
                    mv_sb.append(mvb)

                # aggregate over channel groups: [32, 2] = [mean_g, E[x^2]_g]
                g_ps = pp_sm.tile([GROUPS, 2], f32, tag="den", name="den")
                for t in range(CT):
                    nc.tensor.matmul(g_ps, lhsT=inda_sb[t], rhs=mv_sb[t],
                                     start=(t == 0), stop=(t == CT - 1))
                g_sb = stat_pool.tile([GROUPS, 2], f32, tag="gsb", name="gsb")
                nc.vector.tensor_copy(out=g_sb, in_=g_ps)
                gm2 = stat_pool.tile([GROUPS, 1], f32, tag="gm2", name="gm2")
                nc.vector.tensor_mul(gm2, g_sb[:, 0:1], g_sb[:, 0:1])
                gvar = stat_pool.tile([GROUPS, 1], f32, tag="gvar", name="gvar")
                nc.vector.tensor_sub(gvar, g_sb[:, 1:2], gm2)
                eps_col = stat_pool.tile([GROUPS, 1], f32, tag="eps", name="eps")
                nc.vector.memset(eps_col, EPS)
                gstd = stat_pool.tile([GROUPS, 1], f32, tag="gstd", name="gstd")
                nc.scalar.activation(out=gstd, in_=gvar, func=SQRT, bias=eps_col)
                ga = stat_pool.tile([GROUPS, 1], f32, tag="ga", name="ga")
                nc.vector.reciprocal(out=ga, in_=gstd)
                coeffs = stat_pool.tile([GROUPS, 2], bf16, tag="coef", name="coef")
                nc.vector.tensor_copy(out=coeffs[:, 0:1], in_=ga)
                nc.vector.tensor_copy(out=coeffs[:, 1:2], in_=g_sb[:, 0:1])

                # broadcast group coeffs to per-channel scale/shift columns
                sc_cols = []
                tc_cols = []
                for t in range(CT):
                    b_ps = pp_sm.tile([P, 2], f32, tag="den", name="den")
                    nc.tensor.matmul(b_ps, lhsT=indb_sb[t], rhs=coeffs,
                                     start=True, stop=True)
                    bc = stat_pool.tile([P, 2], f32, tag="bc", name="bc")
                    nc.vector.tensor_copy(out=bc, in_=b_ps)
                    s_col = stat_pool.tile([P, 1], f32, tag="scol", name="scol")
                    nc.vector.tensor_mul(s_col, col_sb["gamma"][t], bc[:, 0:1])
                    tmp = stat_pool.tile([P, 1], f32, tag="tmp", name="tmp")
                    nc.vector.tensor_mul(tmp, bc[:, 1:2], s_col)
                    t_col = stat_pool.tile([P, 1], f32, tag="tcol", name="tcol")
                    nc.vector.tensor_sub(t_col, col_sb["beta"][t], tmp)
                    sc_cols.append(s_col)
                    tc_cols.append(t_col)

                # GroupNorm folding: wk@(s*x+t) = (wk*s)@x + wk@t.  Scale the
                # QKV weights per input channel into fp8 DoubleRow layout
                # [128, 2, C]; the wk@t bias corrections are tiny PE matmuls.
                tcb = []
                for t in range(CT):
                    tb = stat_pool.tile([P, 1], bf16, tag="tcb", name="tcb")
                    nc.vector.tensor_copy(out=tb, in_=tc_cols[t])
                    tcb.append(tb)
                w2 = {}
                for name in ("wkT", "wvT", "wqT"):
                    tiles = [const.tile([P, 2, C], f8, tag=f"{name}s{pr}",
                                        name=f"{name}s{pr}")
                             for pr in range(CP)]
                    for ci in range(CT):
                        dst = tiles[ci // 2][:, ci % 2, :]
                        if ci % 2 == 0:
                            nc.vector.tensor_scalar_mul(dst, w_sb[name][ci],
                                                        sc_cols[ci])
                        else:
                            nc.scalar.activation(out=dst, in_=w_sb[name][ci],
                                                 func=ID, scale=sc_cols[ci])
                    w2[name] = tiles

                # bias corrections: bq2[m] = bq[m] + sum_c wq[d,c] t_c
                # (K needs no bias at all: q_i.(k_j + c) with c constant over
                # j is softmax-invariant, so bk and the wk@t correction drop.)
                bias2 = {}
                for name, bcol in (("wqT", "bq"),):
                    cols2 = []
                    for m in range(CT):
                        tk_ps = pp_sm.tile([P, 1], f32, tag="den", name="den")
                        for ci in range(CT):
                            nc.tensor.matmul(
                                tk_ps,
                                lhsT=w_sb[name][ci][:, m * P:(m + 1) * P],
                                rhs=tcb[ci],
                                start=(ci == 0), stop=(ci == CT - 1))
                        b2 = stat_pool.tile([P, 1], f32, tag=f"b2{name}{m}",
                                            name=f"b2{name}{m}")
                        nc.vector.tensor_scalar(
                            out=b2, in0=tk_ps, scalar1=col_sb[bcol][m],
                            scalar2=None, op0=mybir.AluOpType.add)
                        cols2.append(b2)
                    bias2[name] = cols2
                # v bias row: bvt[c] = bv[c] + sum_c' t_c' wv[c,c'], broadcast
                tv_ps = pp_sm.tile([1, C], f32, tag="den", name="den")
                for ci in range(CT):
                    nc.tensor.matmul(tv_ps, lhsT=tcb[ci], rhs=w_sb["wvT"][ci],
                                     start=(ci == 0), stop=(ci == CT - 1))
                bvt_row = stat_pool.tile([1, C], f32, tag="bvtr", name="bvtr")
                nc.vector.tensor_add(bvt_row, tv_ps, bv_row)
                bvt_ps = pp_av.tile([P, IBS], f32, tag="pav", name="bvtps")
                nc.tensor.matmul(bvt_ps, lhsT=ones_rowf, rhs=bvt_row,
                                 start=True, stop=True)
                bvt_bcast = const.tile([P, C], f32, tag="bvt_bcast",
                                       name="bvt_bcast")
                nc.scalar.activation(out=bvt_bcast, in_=bvt_ps, func=ID)
                worig_cm.__exit__(None, None, None)

                # ---- phase 2: projections straight from fp8 x (DoubleRow) ----
                # K first (bias-free drains -> nothing to wait for), then V
                # (bvt ready by then), then Q (needs the bq2 bias columns).
                for nch in range(NCH):
                    hsl = slice(nch * IBS, (nch + 1) * IBS)
                    for m in range(CT):
                        ps = pp_mm.tile([P, IBS], f32, tag="mm", name="mm")
                        for pr in range(CP):
                            nc.tensor.matmul(
                                ps,
                                lhsT=w2["wkT"][pr][:, :, m * P:(m + 1) * P],
                                rhs=x2_sb[pr][:, :, hsl],
                                start=(pr == 0), stop=(pr == CP - 1),
                                perf_mode=DR)
                        nc.scalar.activation(
                            out=k2_sb[m // 2][:, m % 2, hsl], in_=ps, func=ID)

                # V^T projection; bias-add on DVE drains each PSUM right away
                # v2[jp][p, i, c] = V[c, key (2*jp+i)*128+p]
                v2_sb = []
                for jp in range(JP):
                    v2 = v_pool.tile([P, 2, C], f8, tag="v", name="v")
                    for i in range(2):
                        jt = 2 * jp + i
                        ps = pp_mm.tile([P, C], f32, tag="mm", name="mm")
                        for pr in range(CP):
                            nc.tensor.matmul(
                                ps,
                                lhsT=x2_sb[pr][:, :, jt * P:(jt + 1) * P],
                                rhs=w2["wvT"][pr],
                                start=(pr == 0), stop=(pr == CP - 1),
                                perf_mode=DR)
                        nc.vector.tensor_add(v2[:, i, :], ps, bvt_bcast)
                    v2_sb.append(v2)

                for nch in range(IB):
                    hsl = slice(nch * IBS, (nch + 1) * IBS)
                    for m in range(CT):
                        ps = pp_mm.tile([P, IBS], f32, tag="mm", name="mm")
                        for pr in range(CP):
                            nc.tensor.matmul(
                                ps,
                                lhsT=w2["wqT"][pr][:, :, m * P:(m + 1) * P],
                                rhs=x2_sb[pr][:, :, hsl],
                                start=(pr == 0), stop=(pr == CP - 1),
                                perf_mode=DR)
                        nc.scalar.activation(
                            out=q2_sb[m // 2][:, m % 2, hsl], in_=ps,
                            func=ID, bias=bias2["wqT"][m], scale=1.0)

            # ---- phase 3: attention + output proj + residual ----
            p_pool = ctx.enter_context(tc.tile_pool(name="p", bufs=10))
            xqb_pool = ctx.enter_context(tc.tile_pool(name="xqb", bufs=8))
            a_pool = ctx.enter_context(tc.tile_pool(name="a", bufs=2 * CT))
            o_pool = ctx.enter_context(tc.tile_pool(name="o", bufs=3))
            sm_pool = ctx.enter_context(tc.tile_pool(name="sm", bufs=2))

            LOOKAHEAD = 6  # key-tile pairs

            def emit_scores(bi, jp):
                """exp(score) tile for key pair jp: [p, i, query] fp8."""
                q0, ibs = bi * IBS, IBS
                isl = slice(q0, q0 + ibs)
                p2 = p_pool.tile([P, 2, ibs], f8, tag="p", name="p")
                for i in range(2):
                    jt = 2 * jp + i
                    ps = pp_mm.tile([P, ibs], f32, tag="mm", name="mm")
                    for pr in range(CP):
                        nc.tensor.matmul(
                            ps,
                            lhsT=k2_sb[pr][:, :, jt * P:(jt + 1) * P],
                            rhs=q2_sb[pr][:, :, isl],
                            start=(pr == 0), stop=(pr == CP - 1),
                            perf_mode=DR)
                    nc.scalar.activation(out=p2[:, i, :], in_=ps, func=EXP,
                                         scale=SM_SCALE, bias=ebias)
                return p2

            pending = {}
            NBLK = IB
            for bi in range(NBLK):
                q0, ibs = bi * IBS, IBS
                isl = slice(q0, q0 + ibs)
                nj = ibs // P
                # prefetch the residual tiles early so the o2 adds never wait
                xqb_l = []
                for dt_ in range(CT):
                    xqb_t = xqb_pool.tile([P, ibs], f32, tag="xqb", name="xqb")
                    nc.sync.dma_start(out=xqb_t,
                                      in_=xqb_d[dt_ * P:(dt_ + 1) * P, isl])
                    xqb_l.append(xqb_t)
                pav = [pp_av.tile([P, ibs], f32, tag="pav", name="pav")
                       for _ in range(CT)]
                acc = sm_pool.tile([P, ibs], f32, tag="acc", name="acc")
                accg = sm_pool.tile([P, ibs], f32, tag="accg", name="accg")
                for jp in range(JP):
                    p2 = pending.pop((bi, jp), None)
                    if p2 is None:
                        p2 = emit_scores(bi, jp)
                    # softmax denominator partials, split DVE/GPSIMD
                    if jp == 0:
                        nc.vector.tensor_copy(out=acc, in_=p2[:, 0, :])
                        nc.gpsimd.tensor_copy(out=accg, in_=p2[:, 1, :])
                    else:
                        nc.vector.tensor_add(acc, acc, p2[:, 0, :])
                        nc.gpsimd.tensor_add(accg, accg, p2[:, 1, :])
                    for m in range(CT):
                        nc.tensor.matmul(
                            pav[m],
                            lhsT=v2_sb[jp][:, :, m * P:(m + 1) * P],
                            rhs=p2,
                            start=(jp == 0), stop=(jp == JP - 1),
                            perf_mode=DR)

                nc.vector.tensor_add(acc, acc, accg)

                # denT[qp, j] = sum_p acc[p, j*128+qp]: transposed partition
                # reduce so the DVE reciprocal runs 128-wide (a [1, 512] row
                # would serialize on one DVE lane: ~4us -> ~0.1us).
                denT = pp_sm.tile([P, nj], f32, tag="den", name="den")
                for j in range(nj):
                    nc.tensor.matmul(denT[:, j:j + 1],
                                     lhsT=acc[:, j * P:(j + 1) * P],
                                     rhs=ones_colf, start=True, stop=True)
                recipT = sm_pool.tile([P, nj], f32, tag="recipT",
                                      name="recipT")
                nc.vector.reciprocal(out=recipT, in_=denT)
                tr_ps = pp_sm.tile([nj, P], f32, tag="den", name="den")
                nc.tensor.transpose(tr_ps, recipT, ident_sb)
                recip4 = sm_pool.tile([nj, P], f32, tag="recip4",
                                      name="recip4")
                nc.scalar.activation(out=recip4, in_=tr_ps, func=ID)
                # broadcast across partitions: bc[p, j*128+q] = recip4[j, q]
                # (lives in pp_sm: a pp_av slot would alias pav[0], whose
                # drain now depends on recip_b <- bc_ps -- a WAR cycle)
                bc_ps = pp_sm.tile([P, ibs], f32, tag="den", name="bcps")
                for j in range(nj):
                    nc.tensor.matmul(bc_ps[:, j * P:(j + 1) * P],
                                     lhsT=ej_sb[0:nj, j * P:(j + 1) * P],
                                     rhs=recip4, start=True, stop=True)
                recip_b = sm_pool.tile([P, ibs], f32, tag="recip_b",
                                       name="recip_b")
                nc.scalar.activation(out=recip_b, in_=bc_ps, func=ID)

                # score lookahead into the next block keeps the PE busy while
                # the normalize/O-proj tail of this block resolves; emitted
                # after the recip chain so its exps don't delay recip_b on ACT
                if bi + 1 < NBLK:
                    for la in range(LOOKAHEAD):
                        pending[(bi + 1, la)] = emit_scores(bi + 1, la)

                # normalized attention output straight to fp8 DoubleRow pairs
                # (the 1/den scale applied on the PSUM drain; DVE only -- it
                # reads PSUM)
                a8 = [a_pool.tile([P, 2, ibs], f8, tag="a", name="a")
                      for _ in range(CP)]
                for m in range(CT):
                    nc.vector.tensor_mul(a8[m // 2][:, m % 2, :], pav[m],
                                         recip_b)

                # O-projection (fp8 DoubleRow) + residual
                for dt_ in range(CT):
                    po = pp_mm.tile([P, ibs], f32, tag="mm", name="mm")
                    for pr in range(CP):
                        nc.tensor.matmul(
                            po,
                            lhsT=wo2[pr][:, :, dt_ * P:(dt_ + 1) * P],
                            rhs=a8[pr],
                            start=(pr == 0), stop=(pr == CP - 1),
                            perf_mode=DR)
                    o2 = o_pool.tile([P, ibs], f32, tag="o2", name="o2")
                    nc.vector.tensor_add(o2, po, xqb_l[dt_])
                    # spread the output DMAs over queues so the last block's
                    # stores drain in parallel
                    oeng = (nc.sync, nc.scalar, nc.gpsimd, nc.sync)[dt_]
                    oeng.dma_start(out=out_d[dt_ * P:(dt_ + 1) * P, isl],
                                   in_=o2)

    nc.finalize()
    return nc


def _make_consts():
    """Constant (core-independent) input arrays (packed)."""
    ind_a = np.zeros((P, CT * GROUPS), ml_dtypes.bfloat16)
    ind_b = np.zeros((GROUPS, CT * P), ml_dtypes.bfloat16)
    for t in range(CT):
        for p in range(P):
            g = (t * P + p) // GSIZE
            ind_a[p, t * GROUPS + g] = 1.0 / GSIZE
            ind_b[g, t * P + p] = 1.0
    return ind_a, ind_b


def _make_ej():
    ej = np.zeros((4, 4 * P), np.float32)
    for j in range(4):
        ej[j, j * P:(j + 1) * P] = 1.0
    return ej


def make_in_maps(x, gn_gamma, gn_beta, wq, bq, wk, bk, wv, bv, wo, bo):
    ind_a, ind_b = _make_consts()
    bf = ml_dtypes.bfloat16
    f8 = ml_dtypes.float8_e4m3
    cols = np.stack([np.asarray(a, np.float32) for a in
                     (bq, bk, bv, bo, gn_gamma, gn_beta)], axis=1)
    common = {
        "wqT": np.ascontiguousarray(np.asarray(wq, np.float32).T).astype(bf),
        "wkT": np.ascontiguousarray(np.asarray(wk, np.float32).T).astype(bf),
        "wvT": np.ascontiguousarray(np.asarray(wv, np.float32).T).astype(bf),
        "woT8": np.ascontiguousarray(np.asarray(wo, np.float32).T).astype(f8),
        "cols": np.ascontiguousarray(cols),
        "ind_a": ind_a,
        "ind_b": ind_b,
        "ident": np.eye(P, dtype=np.float32),
        "ej": _make_ej(),
    }
    x = np.asarray(x, np.float32)
    in_maps = []
    for core in range(N_CORES):
        b, half = divmod(core, 2)
        xb = x[b].reshape(C, N)
        xr = np.concatenate(
            [xb[:, half * NQ:(half + 1) * NQ],
             xb[:, (1 - half) * NQ:(2 - half) * NQ]],
            axis=1)
        xqb = xr[:, :NQ] + np.asarray(bo, np.float32).reshape(C, 1)
        in_maps.append({"xr": np.ascontiguousarray(xr).astype(f8),
                        "xqb": np.ascontiguousarray(xqb), **common})
    return in_maps


def gather_out(results):
    out = np.empty((B, C, N), np.float32)
    for core in range(N_CORES):
        b, half = divmod(core, 2)
        out[b][:, half * NQ:(half + 1) * NQ] = results[core]["out"]
    return out.reshape(B, C, H, W)


def get_nc():
    if "nc" not in _cache:
        _cache["nc"] = _build_nc()
    return _cache["nc"]


def kernel(**inputs):
    from concourse.bass_utils import run_bass_kernel_spmd

    nc = get_nc()
    in_maps = make_in_maps(**inputs)
    res = run_bass_kernel_spmd(nc, in_maps, list(range(N_CORES)))
    return gather_out(res.results)


if __name__ == "__main__":
    nc = _build_nc()
    print("built ok:", len(nc.m.functions[0].allocations), "allocations")


# revision 40
# speedup vs baseline: 1.0062x; 1.0062x over previous
"""Trainium2 Bass kernel for AttnBlock (GroupNorm + 1x1-conv QKV self-attention
+ output proj + residual) on x: [4, 512, 64, 64] fp32, distributed over 8
NeuronCores.

Sharding: data-parallel over batch (4) x sequence-parallel over the N=H*W=4096
token axis (2 halves) = 8 cores. Each core receives the full image of its
batch element with the token axis rotated so that its 2048 query tokens come
first; it computes GroupNorm + K/V for all 4096 tokens (duplicated within the
batch pair -- no collectives needed) and Q/attention/output only for its 2048
queries. The host gathers the 8 [512, 2048] outputs back into [4, 512, 64, 64].

All matmuls (QKV projections, scores, attention@V, O-projection) run in
fp8e4m3 with MatmulPerfMode.DoubleRow (2 fp8 weights per PE cell -> 2x
MACs/cycle, ~228ns per [256x128x512] instruction = ~94% of fp8 peak), with
fp32 PSUM accumulation. Key structure:
- x ships in fp8; GroupNorm stats are computed from the fp8 tiles (the
  ~0.1% var bias is negligible) and folded into the projections:
  wk@(s*x+t) = (wk*s)@x + (wk@t). The scaled weights land in fp8 DoubleRow
  layout [128, 2, C].
- The K-projection needs no bias at all: q_i.(k_j + c) with c constant over
  j is softmax-invariant, so bk and the wk@t correction drop. Projection
  order K, V, Q so only the last (Q) waits on its bias columns.
- Scores are computed transposed (S^T = K^T Q per key tile) so softmax and
  the attention@V contraction need no transposes. exp runs on ACT straight
  out of PSUM with bias -2 (keeps exp <= ~70, under fp8e4's 240 max) and
  writes fp8 into the DoubleRow-paired p tiles.
- Softmax denominator partials accumulate on DVE+GPSIMD off the critical
  path; the partition reduce runs transposed (4x [128,1] matmuls) so the
  DVE reciprocal is 128-wide (~0.1us, not a 4us single-lane [1,512] op),
  then a PE transpose + 4 selector matmuls broadcast 1/den to all
  partitions. The normalized attention output drains straight to fp8
  DoubleRow pairs feeding the fp8 O-projection.
- A 6-pair score/exp lookahead across query blocks keeps the PE dense
  through each block's normalize/O-proj/store tail.
- The residual is added from a host-precomputed fp32 x+bo tensor; output
  stores spread across the sync/ACT/GPSIMD DMA queues.
Measured: ~235us HW exec on 8 cores (baseline bf16 version: ~378us),
rel l2 vs fp32 reference ~4.0e-3.
"""

import numpy as np
import ml_dtypes

B, C, H, W = 4, 512, 64, 64
N = H * W            # 4096 tokens
NQ = N // 2          # 2048 queries per core
P = 128              # partitions
CT = C // P          # 4 channel tiles
CP = CT // 2         # 2 channel-tile pairs (DoubleRow)
JT = N // P          # 32 key/token tiles
JP = JT // 2         # 16 key-tile pairs (DoubleRow)
IBS = 512            # query block (free dim of score matmuls)
IB = NQ // IBS       # 4 query blocks per core
NCH = N // IBS       # 8 n-chunks for full-N projections
GROUPS = 32
GSIZE = C // GROUPS  # 16 channels per group
EPS = 1e-6
SM_SCALE = float(C) ** -0.5
EXP_BIAS = -2.0      # exp(s - 2): max score ~6.2 -> exp <= ~70 < 240 fp8e4 max

N_CORES = 8

_cache = {}


def _build_nc():
    import concourse.bass as bass
    import concourse.mybir as mybir
    import concourse.tile as tile
    from concourse import bacc

    f32 = mybir.dt.float32
    bf16 = mybir.dt.bfloat16
    f8 = mybir.dt.float8e4
    ID = mybir.ActivationFunctionType.Identity
    EXP = mybir.ActivationFunctionType.Exp
    SQRT = mybir.ActivationFunctionType.Sqrt
    DR = mybir.MatmulPerfMode.DoubleRow

    nc = bacc.Bacc("TRN2")

    xr_d = nc.declare_dram_parameter("xr", [C, N], f8, isOutput=False)
    w_d = {
        name: nc.declare_dram_parameter(name, [C, C], bf16, isOutput=False)
        for name in ("wqT", "wkT", "wvT")
    }
    wo8_d = nc.declare_dram_parameter("woT8", [C, C], f8, isOutput=False)
    cols_d = nc.declare_dram_parameter("cols", [C, 6], f32, isOutput=False)
    xqb_d = nc.declare_dram_parameter("xqb", [C, NQ], f32, isOutput=False)
    id_d = nc.declare_dram_parameter("ident", [P, P], f32, isOutput=False)
    ej_d = nc.declare_dram_parameter("ej", [4, 4 * P], f32, isOutput=False)
    inda_d = nc.declare_dram_parameter("ind_a", [P, CT * GROUPS], bf16, isOutput=False)
    indb_d = nc.declare_dram_parameter("ind_b", [GROUPS, CT * P], bf16, isOutput=False)
    out_d = nc.declare_dram_parameter("out", [C, NQ], f32, isOutput=True)

    with tile.TileContext(nc) as tc:
        from contextlib import ExitStack

        with ExitStack() as ctx:
            const = ctx.enter_context(tc.tile_pool(name="const", bufs=1))
            pp_mm = ctx.enter_context(tc.tile_pool(name="pp_mm", bufs=3, space="PSUM"))
            pp_av = ctx.enter_context(tc.tile_pool(name="pp_av", bufs=4, space="PSUM"))
            pp_sm = ctx.enter_context(tc.tile_pool(name="pp_sm", bufs=1, space="PSUM"))

            # ---- batched small constants (few DMAs; issued after x) ----
            cols_t = [const.tile([P, 6], f32, tag=f"cols{t}", name=f"cols{t}")
                      for t in range(CT)]
            inda_t = const.tile([P, CT * GROUPS], bf16, tag="inda", name="inda")
            indb_t = const.tile([GROUPS, CT * P], bf16, tag="indb", name="indb")
            col_sb = {nm: [cols_t[t][:, i:i + 1] for t in range(CT)]
                      for i, nm in enumerate(("bq", "bk", "bv", "bo",
                                              "gamma", "beta"))}
            inda_sb = [inda_t[:, t * GROUPS:(t + 1) * GROUPS] for t in range(CT)]
            indb_sb = [indb_t[:, t * P:(t + 1) * P] for t in range(CT)]

            ones_colf = const.tile([P, 1], f32, tag="ones_colf", name="ones_colf")
            nc.vector.memset(ones_colf, 1.0)
            ones_rowf = const.tile([1, P], f32, tag="ones_rowf", name="ones_rowf")
            nc.vector.memset(ones_rowf, 1.0)
            ebias = const.tile([P, 1], f32, tag="ebias", name="ebias")
            nc.vector.memset(ebias, EXP_BIAS)
            ident_sb = const.tile([P, P], f32, tag="ident", name="ident")
            ej_sb = const.tile([4, 4 * P], f32, tag="ej", name="ej")

            stat_pool = ctx.enter_context(tc.tile_pool(name="stat", bufs=4 * CT))

            k_pool = ctx.enter_context(tc.tile_pool(name="k", bufs=CP))
            v_pool = ctx.enter_context(tc.tile_pool(name="v", bufs=JP))
            q_pool = ctx.enter_context(tc.tile_pool(name="q", bufs=CP))
            # DoubleRow-paired K/Q: [p, i, j] = val[ch = pair*256 + i*128 + p, j]
            k2_sb = [k_pool.tile([P, 2, N], f8, tag="k", name="k")
                     for _ in range(CP)]
            q2_sb = [q_pool.tile([P, 2, NQ], f8, tag="q", name="q")
                     for _ in range(CP)]

            # ---- phase 1: x load (2 HW-DGE queues) + GroupNorm stats ----
            # stats for tiles 0,2,3 via DVE bn_stats; tile 1 via ACT
            # Square/Identity with accum_out (free-dim sums) to halve the
            # serial DVE chain on the critical path.
            mv_sb = []
            with tc.tile_pool(name="xr", bufs=CP) as xr_pool:
                # [p, i, n] = x[ch = pair*256 + i*128 + p, n] in fp8
                x2_sb = [xr_pool.tile([P, 2, N], f8, tag="xr", name="xr")
                         for _ in range(CP)]
                xsl = [x2_sb[t // 2][:, t % 2, :] for t in range(CT)]
                st_sb = []
                acc_cols = []
                # tile 0 lands as 512-wide chunks so the first bn_stats can
                # start ~2us earlier; sync feeds DVE's tiles 0 then 2, scalar
                # feeds ACT's tile 1 first so its slower chain starts early,
                # then DVE's tile 3
                order = [(0, slice(ch * 512, (ch + 1) * 512), nc.sync)
                         for ch in range(4)]
                order += [(t, slice(ch * (N // 4), (ch + 1) * (N // 4)), eng)
                          for t, ch, eng in
                          [(1, 0, nc.scalar), (1, 1, nc.scalar),
                           (0, 2, nc.sync), (0, 3, nc.sync),
                           (1, 2, nc.scalar), (1, 3, nc.scalar),
                           (2, 0, nc.sync), (2, 1, nc.sync),
                           (3, 0, nc.scalar), (3, 1, nc.scalar),
                           (2, 2, nc.sync), (2, 3, nc.sync),
                           (3, 2, nc.scalar), (3, 3, nc.scalar)]]
                for t, csl, eng in order:
                    eng.dma_start(out=x2_sb[t // 2][:, t % 2, csl],
                                  in_=xr_d[t * P:(t + 1) * P, csl])

                # HAM warm-up: ~56 junk fp8 matmuls bridge the stats-bound
                # head (PE otherwise idle +10..+25us) so the PE clock gate is
                # already at 8/8 when the real projection stream begins.
                # Sized to finish just before the stats matmuls even if the
                # first ~7us runs cold.
                junk_ps = pp_sm.tile([P, IBS], f32, tag="den", name="junk")
                for _ in range(72):
                    nc.tensor.matmul(junk_ps,
                                     lhsT=x2_sb[0][:, 0, 0:P],
                                     rhs=x2_sb[0][:, 0, 0:IBS],
                                     start=True, stop=True)
                for t in range(CT):
                    xt_g = xsl[t].rearrange("p (s f) -> p s f", f=512)
                    if t != 1:
                        st = stat_pool.tile([P, N // 512, 6], f32, tag="bnst",
                                            name="bnst")
                        sums = None
                        for s in range(N // 512):
                            nc.vector.bn_stats(out=st[:, s, :],
                                               in_=xt_g[:, s, :])
                    else:
                        st = None
                        sums = stat_pool.tile([P, 2, N // 512], f32, tag="acs",
                                              name="acs")
                        for s in range(N // 512):
                            scr = stat_pool.tile([P, 512], bf16, tag="scr",
                                                 name="scr", bufs=2)
                            nc.scalar.activation(
                                out=scr, in_=xt_g[:, s, :],
                                func=mybir.ActivationFunctionType.Square,
                                accum_out=sums[:, 1, s:s + 1])
                            nc.scalar.activation(
                                out=scr, in_=xt_g[:, s, :], func=ID,
                                accum_out=sums[:, 0, s:s + 1])
                    st_sb.append(st)
                    acc_cols.append(sums)

                # batched consts + weights + bv now (queues free after x)
                nc.sync.dma_start(out=inda_t, in_=inda_d[:, :])
                nc.sync.dma_start(out=indb_t, in_=indb_d[:, :])
                nc.sync.dma_start(out=ident_sb, in_=id_d[:, :])
                nc.sync.dma_start(out=ej_sb, in_=ej_d[:, :])
                for t in range(CT):
                    nc.sync.dma_start(out=cols_t[t],
                                      in_=cols_d[t * P:(t + 1) * P, :])
                worig_cm = tc.tile_pool(name="worig", bufs=1)
                worig_pool = worig_cm.__enter__()
                w_sb = {}
                for name in ("wkT", "wqT", "wvT"):
                    tiles = []
                    for t in range(CT):
                        tw = worig_pool.tile([P, C], bf16, tag=f"{name}{t}",
                                             name=f"{name}{t}")
                        nc.sync.dma_start(out=tw,
                                          in_=w_d[name][t * P:(t + 1) * P, :])
                        tiles.append(tw)
                    w_sb[name] = tiles
                # O-projection weights, fp8 DoubleRow layout (host-cast)
                wo2 = [const.tile([P, 2, C], f8, tag=f"wo2{pr}",
                                  name=f"wo2{pr}") for pr in range(CP)]
                for pr in range(CP):
                    for i in range(2):
                        nc.sync.dma_start(
                            out=wo2[pr][:, i, :],
                            in_=wo8_d[(2 * pr + i) * P:(2 * pr + i + 1) * P, :])
                bv_row = const.tile([1, C], f32, tag="bv_row", name="bv_row")
                nc.sync.dma_start(
                    out=bv_row,
                    in_=cols_d[:, 2:3].rearrange("c one -> one c"))

                for t in range(CT):
                    mv = stat_pool.tile([P, 2], f32, tag="mv", name="mv")
                    if st_sb[t] is not None:
                        nc.vector.bn_aggr(out=mv, in_=st_sb[t])
                        # mv = [mean, var] -> [mean, E[x^2]]
                        msq = stat_pool.tile([P, 1], f32, tag="msq", name="msq")
                        nc.vector.tensor_mul(msq, mv[:, 0:1], mv[:, 0:1])
                        nc.vector.tensor_add(mv[:, 1:2], mv[:, 1:2], msq)
                    else:
                        # sums[:, s, 0]=sum(x), [:, s, 1]=sum(x^2) per 512-chunk
                        sred = stat_pool.tile([P, 2], f32, tag="sred", name="sred")
                        nc.vector.tensor_reduce(
                            out=sred, in_=acc_cols[t],
                            op=mybir.AluOpType.add, axis=mybir.AxisListType.X)
                        nc.vector.tensor_scalar_mul(mv, sred, 1.0 / N)
                    mvb = stat_pool.tile([P, 2], bf16, tag="mvb", name="mvb")
                    nc.vector.tensor_copy(out=mvb, in_=mv)
                    mv_sb.append(mvb)

                # aggregate over channel groups: [32, 2] = [mean_g, E[x^2]_g]
                g_ps = pp_sm.tile([GROUPS, 2], f32, tag="den", name="den")
                for t in range(CT):
                    nc.tensor.matmul(g_ps, lhsT=inda_sb[t], rhs=mv_sb[t],
                                     start=(t == 0), stop=(t == CT - 1))
                g_sb = stat_pool.tile([GROUPS, 2], f32, tag="gsb", name="gsb")
                nc.vector.tensor_copy(out=g_sb, in_=g_ps)
                gm2 = stat_pool.tile([GROUPS, 1], f32, tag="gm2", name="gm2")
                nc.vector.tensor_mul(gm2, g_sb[:, 0:1], g_sb[:, 0:1])
                gvar = stat_pool.tile([GROUPS, 1], f32, tag="gvar", name="gvar")
                nc.vector.tensor_sub(gvar, g_sb[:, 1:2], gm2)
                eps_col = stat_pool.tile([GROUPS, 1], f32, tag="eps", name="eps")
                nc.vector.memset(eps_col, EPS)
                gstd = stat_pool.tile([GROUPS, 1], f32, tag="gstd", name="gstd")
                nc.scalar.activation(out=gstd, in_=gvar, func=SQRT, bias=eps_col)
                ga = stat_pool.tile([GROUPS, 1], f32, tag="ga", name="ga")
                nc.vector.reciprocal(out=ga, in_=gstd)
                coeffs = stat_pool.tile([GROUPS, 2], bf16, tag="coef", name="coef")
                nc.vector.tensor_copy(out=coeffs[:, 0:1], in_=ga)
                nc.vector.tensor_copy(out=coeffs[:, 1:2], in_=g_sb[:, 0:1])

                # broadcast group coeffs to per-channel scale/shift columns
                sc_cols = []
                tc_cols = []
                for t in range(CT):
                    b_ps = pp_sm.tile([P, 2], f32, tag="den", name="den")
                    nc.tensor.matmul(b_ps, lhsT=indb_sb[t], rhs=coeffs,
                                     start=True, stop=True)
                    bc = stat_pool.tile([P, 2], f32, tag="bc", name="bc")
                    nc.vector.tensor_copy(out=bc, in_=b_ps)
                    s_col = stat_pool.tile([P, 1], f32, tag="scol", name="scol")
                    nc.vector.tensor_mul(s_col, col_sb["gamma"][t], bc[:, 0:1])
                    tmp = stat_pool.tile([P, 1], f32, tag="tmp", name="tmp")
                    nc.vector.tensor_mul(tmp, bc[:, 1:2], s_col)
                    t_col = stat_pool.tile([P, 1], f32, tag="tcol", name="tcol")
                    nc.vector.tensor_sub(t_col, col_sb["beta"][t], tmp)
                    sc_cols.append(s_col)
                    tc_cols.append(t_col)

                # GroupNorm folding: wk@(s*x+t) = (wk*s)@x + wk@t.  Scale the
                # QKV weights per input channel into fp8 DoubleRow layout
                # [128, 2, C]; the wk@t bias corrections are tiny PE matmuls.
                tcb = []
                for t in range(CT):
                    tb = stat_pool.tile([P, 1], bf16, tag="tcb", name="tcb")
                    nc.vector.tensor_copy(out=tb, in_=tc_cols[t])
                    tcb.append(tb)
                w2 = {}
                for name in ("wkT", "wvT", "wqT"):
                    tiles = [const.tile([P, 2, C], f8, tag=f"{name}s{pr}",
                                        name=f"{name}s{pr}")
                             for pr in range(CP)]
                    for ci in range(CT):
                        dst = tiles[ci // 2][:, ci % 2, :]
                        if ci % 2 == 0:
                            nc.vector.tensor_scalar_mul(dst, w_sb[name][ci],
                                                        sc_cols[ci])
                        else:
                            nc.scalar.activation(out=dst, in_=w_sb[name][ci],
                                                 func=ID, scale=sc_cols[ci])
                    w2[name] = tiles

                # bias corrections: bq2[m] = bq[m] + sum_c wq[d,c] t_c
                # (K needs no bias at all: q_i.(k_j + c) with c constant over
                # j is softmax-invariant, so bk and the wk@t correction drop.)
                bias2 = {}
                for name, bcol in (("wqT", "bq"),):
                    cols2 = []
                    for m in range(CT):
                        tk_ps = pp_sm.tile([P, 1], f32, tag="den", name="den")
                        for ci in range(CT):
                            nc.tensor.matmul(
                                tk_ps,
                                lhsT=w_sb[name][ci][:, m * P:(m + 1) * P],
                                rhs=tcb[ci],
                                start=(ci == 0), stop=(ci == CT - 1))
                        b2 = stat_pool.tile([P, 1], f32, tag=f"b2{name}{m}",
                                            name=f"b2{name}{m}")
                        nc.vector.tensor_scalar(
                            out=b2, in0=tk_ps, scalar1=col_sb[bcol][m],
                            scalar2=None, op0=mybir.AluOpType.add)
                        cols2.append(b2)
                    bias2[name] = cols2
                # v bias row: bvt[c] = bv[c] + sum_c' t_c' wv[c,c'], broadcast
                tv_ps = pp_sm.tile([1, C], f32, tag="den", name="den")
                for ci in range(CT):
                    nc.tensor.matmul(tv_ps, lhsT=tcb[ci], rhs=w_sb["wvT"][ci],
                                     start=(ci == 0), stop=(ci == CT - 1))
                bvt_row = stat_pool.tile([1, C], f32, tag="bvtr", name="bvtr")
                nc.vector.tensor_add(bvt_row, tv_ps, bv_row)
                bvt_ps = pp_av.tile([P, IBS], f32, tag="pav", name="bvtps")
                nc.tensor.matmul(bvt_ps, lhsT=ones_rowf, rhs=bvt_row,
                                 start=True, stop=True)
                bvt_bcast = const.tile([P, C], f32, tag="bvt_bcast",
                                       name="bvt_bcast")
                nc.scalar.activation(out=bvt_bcast, in_=bvt_ps, func=ID)
                worig_cm.__exit__(None, None, None)

                # ---- phase 2: projections straight from fp8 x (DoubleRow) ----
                # K first (bias-free drains -> nothing to wait for), then V
                # (bvt ready by then), then Q (needs the bq2 bias columns).
                for nch in range(NCH):
                    hsl = slice(nch * IBS, (nch + 1) * IBS)
                    for m in range(CT):
                        ps = pp_mm.tile([P, IBS], f32, tag="mm", name="mm")
                        for pr in range(CP):
                            nc.tensor.matmul(
                                ps,
                                lhsT=w2["wkT"][pr][:, :, m * P:(m + 1) * P],
                                rhs=x2_sb[pr][:, :, hsl],
                                start=(pr == 0), stop=(pr == CP - 1),
                                perf_mode=DR)
                        nc.scalar.activation(
                            out=k2_sb[m // 2][:, m % 2, hsl], in_=ps, func=ID)

                # V^T projection; bias-add on DVE drains each PSUM right away
                # v2[jp][p, i, c] = V[c, key (2*jp+i)*128+p]
                v2_sb = []
                for jp in range(JP):
                    v2 = v_pool.tile([P, 2, C], f8, tag="v", name="v")
                    for i in range(2):
                        jt = 2 * jp + i
                        ps = pp_mm.tile([P, C], f32, tag="mm", name="mm")
                        for pr in range(CP):
                            nc.tensor.matmul(
                                ps,
                                lhsT=x2_sb[pr][:, :, jt * P:(jt + 1) * P],
                                rhs=w2["wvT"][pr],
                                start=(pr == 0), stop=(pr == CP - 1),
                                perf_mode=DR)
                        nc.vector.tensor_add(v2[:, i, :], ps, bvt_bcast)
                    v2_sb.append(v2)

                for nch in range(IB):
                    hsl = slice(nch * IBS, (nch + 1) * IBS)
                    for m in range(CT):
                        ps = pp_mm.tile([P, IBS], f32, tag="mm", name="mm")
                        for pr in range(CP):
                            nc.tensor.matmul(
                                ps,
                                lhsT=w2["wqT"][pr][:, :, m * P:(m + 1) * P],
                                rhs=x2_sb[pr][:, :, hsl],
                                start=(pr == 0), stop=(pr == CP - 1),
                                perf_mode=DR)
                        nc.scalar.activation(
                            out=q2_sb[m // 2][:, m % 2, hsl], in_=ps,
                            func=ID, bias=bias2["wqT"][m], scale=1.0)

            # ---- phase 3: attention + output proj + residual ----
            p_pool = ctx.enter_context(tc.tile_pool(name="p", bufs=10))
            xqb_pool = ctx.enter_context(tc.tile_pool(name="xqb", bufs=8))
            a_pool = ctx.enter_context(tc.tile_pool(name="a", bufs=2 * CT))
            o_pool = ctx.enter_context(tc.tile_pool(name="o", bufs=3))
            sm_pool = ctx.enter_context(tc.tile_pool(name="sm", bufs=2))

            LOOKAHEAD = 6  # key-tile pairs

            def emit_scores(bi, jp):
                """exp(score) tile for key pair jp: [p, i, query] fp8."""
                q0, ibs = bi * IBS, IBS
                isl = slice(q0, q0 + ibs)
                p2 = p_pool.tile([P, 2, ibs], f8, tag="p", name="p")
                for i in range(2):
                    jt = 2 * jp + i
                    ps = pp_mm.tile([P, ibs], f32, tag="mm", name="mm")
                    for pr in range(CP):
                        nc.tensor.matmul(
                            ps,
                            lhsT=k2_sb[pr][:, :, jt * P:(jt + 1) * P],
                            rhs=q2_sb[pr][:, :, isl],
                            start=(pr == 0), stop=(pr == CP - 1),
                            perf_mode=DR)
                    nc.scalar.activation(out=p2[:, i, :], in_=ps, func=EXP,
                                         scale=SM_SCALE, bias=ebias)
                return p2

            pending = {}
            NBLK = IB
            for bi in range(NBLK):
                q0, ibs = bi * IBS, IBS
                isl = slice(q0, q0 + ibs)
                nj = ibs // P
                # prefetch the residual tiles early so the o2 adds never wait
                xqb_l = []
                for dt_ in range(CT):
                    xqb_t = xqb_pool.tile([P, ibs], f32, tag="xqb", name="xqb")
                    nc.sync.dma_start(out=xqb_t,
                                      in_=xqb_d[dt_ * P:(dt_ + 1) * P, isl])
                    xqb_l.append(xqb_t)
                pav = [pp_av.tile([P, ibs], f32, tag="pav", name="pav")
                       for _ in range(CT)]
                acc = sm_pool.tile([P, ibs], f32, tag="acc", name="acc")
                accg = sm_pool.tile([P, ibs], f32, tag="accg", name="accg")
                for jp in range(JP):
                    p2 = pending.pop((bi, jp), None)
                    if p2 is None:
                        p2 = emit_scores(bi, jp)
                    # softmax denominator partials, split DVE/GPSIMD
                    if jp == 0:
                        nc.vector.tensor_copy(out=acc, in_=p2[:, 0, :])
                        nc.gpsimd.tensor_copy(out=accg, in_=p2[:, 1, :])
                    else:
                        nc.vector.tensor_add(acc, acc, p2[:, 0, :])
                        nc.gpsimd.tensor_add(accg, accg, p2[:, 1, :])
                    for m in range(CT):
                        nc.tensor.matmul(
                            pav[m],
                            lhsT=v2_sb[jp][:, :, m * P:(m + 1) * P],
                            rhs=p2,
                            start=(jp == 0), stop=(jp == JP - 1),
                            perf_mode=DR)

                nc.vector.tensor_add(acc, acc, accg)

                # denT[qp, j] = sum_p acc[p, j*128+qp]: transposed partition
                # reduce so the DVE reciprocal runs 128-wide (a [1, 512] row
                # would serialize on one DVE lane: ~4us -> ~0.1us).
                denT = pp_sm.tile([P, nj], f32, tag="den", name="den")
                for j in range(nj):
                    nc.tensor.matmul(denT[:, j:j + 1],
                                     lhsT=acc[:, j * P:(j + 1) * P],
                                     rhs=ones_colf, start=True, stop=True)
                recipT = sm_pool.tile([P, nj], f32, tag="recipT",
                                      name="recipT")
                nc.vector.reciprocal(out=recipT, in_=denT)
                tr_ps = pp_sm.tile([nj, P], f32, tag="den", name="den")
                nc.tensor.transpose(tr_ps, recipT, ident_sb)
                recip4 = sm_pool.tile([nj, P], f32, tag="recip4",
                                      name="recip4")
                nc.scalar.activation(out=recip4, in_=tr_ps, func=ID)
                # broadcast across partitions: bc[p, j*128+q] = recip4[j, q]
                # (lives in pp_sm: a pp_av slot would alias pav[0], whose
                # drain now depends on recip_b <- bc_ps -- a WAR cycle)
                bc_ps = pp_sm.tile([P, ibs], f32, tag="den", name="bcps")
                for j in range(nj):
                    nc.tensor.matmul(bc_ps[:, j * P:(j + 1) * P],
                                     lhsT=ej_sb[0:nj, j * P:(j + 1) * P],
                                     rhs=recip4, start=True, stop=True)
                recip_b = sm_pool.tile([P, ibs], f32, tag="recip_b",
                                       name="recip_b")
                nc.scalar.activation(out=recip_b, in_=bc_ps, func=ID)

                # score lookahead into the next block keeps the PE busy while
                # the normalize/O-proj tail of this block resolves; emitted
                # after the recip chain so its exps don't delay recip_b on ACT
                if bi + 1 < NBLK:
                    for la in range(LOOKAHEAD):
                        pending[(bi + 1, la)] = emit_scores(bi + 1, la)

                # normalized attention output straight to fp8 DoubleRow pairs
                # (the 1/den scale applied on the PSUM drain; DVE only -- it
                # reads PSUM)
                a8 = [a_pool.tile([P, 2, ibs], f8, tag="a", name="a")
                      for _ in range(CP)]
                for m in range(CT):
                    nc.vector.tensor_mul(a8[m // 2][:, m % 2, :], pav[m],
                                         recip_b)

                # O-projection (fp8 DoubleRow) + residual
                for dt_ in range(CT):
                    po = pp_mm.tile([P, ibs], f32, tag="mm", name="mm")
                    for pr in range(CP):
                        nc.tensor.matmul(
                            po,
                            lhsT=wo2[pr][:, :, dt_ * P:(dt_ + 1) * P],
                            rhs=a8[pr],
                            start=(pr == 0), stop=(pr == CP - 1),
                            perf_mode=DR)
                    o2 = o_pool.tile([P, ibs], f32, tag="o2", name="o2")
                    nc.vector.tensor_add(o2, po, xqb_l[dt_])
                    # spread the output DMAs over queues so the last block's
                    # stores drain in parallel
                    oeng = (nc.sync, nc.scalar, nc.gpsimd, nc.sync)[dt_]
                    oeng.dma_start(out=out_d[dt_ * P:(dt_ + 1) * P, isl],
                                   in_=o2)

    nc.finalize()
    return nc


def _make_consts():
    """Constant (core-independent) input arrays (packed)."""
    ind_a = np.zeros((P, CT * GROUPS), ml_dtypes.bfloat16)
    ind_b = np.zeros((GROUPS, CT * P), ml_dtypes.bfloat16)
    for t in range(CT):
        for p in range(P):
            g = (t * P + p) // GSIZE
            ind_a[p, t * GROUPS + g] = 1.0 / GSIZE
            ind_b[g, t * P + p] = 1.0
    return ind_a, ind_b


def _make_ej():
    ej = np.zeros((4, 4 * P), np.float32)
    for j in range(4):
        ej[j, j * P:(j + 1) * P] = 1.0
    return ej


def make_in_maps(x, gn_gamma, gn_beta, wq, bq, wk, bk, wv, bv, wo, bo):
    ind_a, ind_b = _make_consts()
    bf = ml_dtypes.bfloat16
    f8 = ml_dtypes.float8_e4m3
    cols = np.stack([np.asarray(a, np.float32) for a in
                     (bq, bk, bv, bo, gn_gamma, gn_beta)], axis=1)
    common = {
        "wqT": np.ascontiguousarray(np.asarray(wq, np.float32).T).astype(bf),
        "wkT": np.ascontiguousarray(np.asarray(wk, np.float32).T).astype(bf),
        "wvT": np.ascontiguousarray(np.asarray(wv, np.float32).T).astype(bf),
        "woT8": np.ascontiguousarray(np.asarray(wo, np.float32).T).astype(f8),
        "cols": np.ascontiguousarray(cols),
        "ind_a": ind_a,
        "ind_b": ind_b,
        "ident": np.eye(P, dtype=np.float32),
        "ej": _make_ej(),
    }
    x = np.asarray(x, np.float32)
    in_maps = []
    for core in range(N_CORES):
        b, half = divmod(core, 2)
        xb = x[b].reshape(C, N)
        xr = np.concatenate(
            [xb[:, half * NQ:(half + 1) * NQ],
             xb[:, (1 - half) * NQ:(2 - half) * NQ]],
            axis=1)
        xqb = xr[:, :NQ] + np.asarray(bo, np.float32).reshape(C, 1)
        in_maps.append({"xr": np.ascontiguousarray(xr).astype(f8),
                        "xqb": np.ascontiguousarray(xqb), **common})
    return in_maps


def gather_out(results):
    out = np.empty((B, C, N), np.float32)
    for core in range(N_CORES):
        b, half = divmod(core, 2)
        out[b][:, half * NQ:(half + 1) * NQ] = results[core]["out"]
    return out.reshape(B, C, H, W)


def get_nc():
    if "nc" not in _cache:
        _cache["nc"] = _build_nc()
    return _cache["nc"]


def kernel(**inputs):
    from concourse.bass_utils import run_bass_kernel_spmd

    nc = get_nc()
    in_maps = make_in_maps(**inputs)
    res = run_bass_kernel_spmd(nc, in_maps, list(range(N_CORES)))
    return gather_out(res.results)


if __name__ == "__main__":
    nc = _build_nc()
    print("built ok:", len(nc.m.functions[0].allocations), "allocations")


# revision 41
# speedup vs baseline: 1.0202x; 1.0139x over previous
"""Trainium2 Bass kernel for AttnBlock (GroupNorm + 1x1-conv QKV self-attention
+ output proj + residual) on x: [4, 512, 64, 64] fp32, distributed over 8
NeuronCores.

Sharding: data-parallel over batch (4) x sequence-parallel over the N=H*W=4096
token axis (2 halves) = 8 cores. Each core receives the full image of its
batch element with the token axis rotated so that its 2048 query tokens come
first; it computes GroupNorm + K/V for all 4096 tokens (duplicated within the
batch pair -- no collectives needed) and Q/attention/output only for its 2048
queries. The host gathers the 8 [512, 2048] outputs back into [4, 512, 64, 64].

All matmuls (QKV projections, scores, attention@V, O-projection) run in
fp8e4m3 with MatmulPerfMode.DoubleRow (2 fp8 weights per PE cell -> 2x
MACs/cycle, ~228ns per [256x128x512] instruction = ~94% of fp8 peak), with
fp32 PSUM accumulation. Key structure:
- x ships in fp8; GroupNorm stats are computed from the fp8 tiles (the
  ~0.1% var bias is negligible) and folded into the projections:
  wk@(s*x+t) = (wk*s)@x + (wk@t). The scaled weights land in fp8 DoubleRow
  layout [128, 2, C].
- The K-projection needs no bias at all: q_i.(k_j + c) with c constant over
  j is softmax-invariant, so bk and the wk@t correction drop. Projection
  order K, V, Q so only the last (Q) waits on its bias columns.
- Scores are computed transposed (S^T = K^T Q per key tile) so softmax and
  the attention@V contraction need no transposes. exp runs on ACT straight
  out of PSUM with bias -2 (keeps exp <= ~70, under fp8e4's 240 max) and
  writes fp8 into the DoubleRow-paired p tiles.
- Softmax denominator partials accumulate on DVE+GPSIMD off the critical
  path; the partition reduce runs transposed (4x [128,1] matmuls) so the
  DVE reciprocal is 128-wide (~0.1us, not a 4us single-lane [1,512] op),
  then a PE transpose + 4 selector matmuls broadcast 1/den to all
  partitions. The normalized attention output drains straight to fp8
  DoubleRow pairs feeding the fp8 O-projection.
- A 6-pair score/exp lookahead across query blocks keeps the PE dense
  through each block's normalize/O-proj/store tail.
- The residual is added from a host-precomputed fp32 x+bo tensor; output
  stores spread across the sync/ACT/GPSIMD DMA queues.
Measured: ~235us HW exec on 8 cores (baseline bf16 version: ~378us),
rel l2 vs fp32 reference ~4.0e-3.
"""

import numpy as np
import ml_dtypes

B, C, H, W = 4, 512, 64, 64
N = H * W            # 4096 tokens
NQ = N // 2          # 2048 queries per core
P = 128              # partitions
CT = C // P          # 4 channel tiles
CP = CT // 2         # 2 channel-tile pairs (DoubleRow)
JT = N // P          # 32 key/token tiles
JP = JT // 2         # 16 key-tile pairs (DoubleRow)
IBS = 512            # query block (free dim of score matmuls)
IB = NQ // IBS       # 4 query blocks per core
NCH = N // IBS       # 8 n-chunks for full-N projections
GROUPS = 32
GSIZE = C // GROUPS  # 16 channels per group
EPS = 1e-6
SM_SCALE = float(C) ** -0.5
EXP_BIAS = -2.0      # exp(s - 2): max score ~6.2 -> exp <= ~70 < 240 fp8e4 max

N_CORES = 8

_cache = {}


def _build_nc():
    import concourse.bass as bass
    import concourse.mybir as mybir
    import concourse.tile as tile
    from concourse import bacc

    f32 = mybir.dt.float32
    bf16 = mybir.dt.bfloat16
    f8 = mybir.dt.float8e4
    ID = mybir.ActivationFunctionType.Identity
    EXP = mybir.ActivationFunctionType.Exp
    SQRT = mybir.ActivationFunctionType.Sqrt
    DR = mybir.MatmulPerfMode.DoubleRow

    nc = bacc.Bacc("TRN2")

    xr_d = nc.declare_dram_parameter("xr", [C, N], f8, isOutput=False)
    w_d = {
        name: nc.declare_dram_parameter(name, [C, C], bf16, isOutput=False)
        for name in ("wqT", "wkT", "wvT")
    }
    wo8_d = nc.declare_dram_parameter("woT8", [C, C], f8, isOutput=False)
    cols_d = nc.declare_dram_parameter("cols", [C, 6], f32, isOutput=False)
    xqb_d = nc.declare_dram_parameter("xqb", [C, NQ], f32, isOutput=False)
    id_d = nc.declare_dram_parameter("ident", [P, P], f32, isOutput=False)
    ej_d = nc.declare_dram_parameter("ej", [4, 4 * P], f32, isOutput=False)
    inda_d = nc.declare_dram_parameter("ind_a", [P, CT * GROUPS], bf16, isOutput=False)
    indb_d = nc.declare_dram_parameter("ind_b", [GROUPS, CT * P], bf16, isOutput=False)
    out_d = nc.declare_dram_parameter("out", [C, NQ], f32, isOutput=True)

    with tile.TileContext(nc) as tc:
        from contextlib import ExitStack

        with ExitStack() as ctx:
            const = ctx.enter_context(tc.tile_pool(name="const", bufs=1))
            pp_mm = ctx.enter_context(tc.tile_pool(name="pp_mm", bufs=3, space="PSUM"))
            pp_av = ctx.enter_context(tc.tile_pool(name="pp_av", bufs=4, space="PSUM"))
            pp_sm = ctx.enter_context(tc.tile_pool(name="pp_sm", bufs=1, space="PSUM"))

            # ---- batched small constants (few DMAs; issued after x) ----
            cols_t = [const.tile([P, 6], f32, tag=f"cols{t}", name=f"cols{t}")
                      for t in range(CT)]
            inda_t = const.tile([P, CT * GROUPS], bf16, tag="inda", name="inda")
            indb_t = const.tile([GROUPS, CT * P], bf16, tag="indb", name="indb")
            col_sb = {nm: [cols_t[t][:, i:i + 1] for t in range(CT)]
                      for i, nm in enumerate(("bq", "bk", "bv", "bo",
                                              "gamma", "beta"))}
            inda_sb = [inda_t[:, t * GROUPS:(t + 1) * GROUPS] for t in range(CT)]
            indb_sb = [indb_t[:, t * P:(t + 1) * P] for t in range(CT)]

            ones_colf = const.tile([P, 1], f32, tag="ones_colf", name="ones_colf")
            nc.vector.memset(ones_colf, 1.0)
            ones_rowf = const.tile([1, P], f32, tag="ones_rowf", name="ones_rowf")
            nc.vector.memset(ones_rowf, 1.0)
            ebias = const.tile([P, 1], f32, tag="ebias", name="ebias")
            nc.vector.memset(ebias, EXP_BIAS)
            ident_sb = const.tile([P, P], f32, tag="ident", name="ident")
            ej_sb = const.tile([4, 4 * P], f32, tag="ej", name="ej")

            stat_pool = ctx.enter_context(tc.tile_pool(name="stat", bufs=4 * CT))

            k_pool = ctx.enter_context(tc.tile_pool(name="k", bufs=CP))
            v_pool = ctx.enter_context(tc.tile_pool(name="v", bufs=JP))
            q_pool = ctx.enter_context(tc.tile_pool(name="q", bufs=CP))
            # DoubleRow-paired K/Q: [p, i, j] = val[ch = pair*256 + i*128 + p, j]
            k2_sb = [k_pool.tile([P, 2, N], f8, tag="k", name="k")
                     for _ in range(CP)]
            q2_sb = [q_pool.tile([P, 2, NQ], f8, tag="q", name="q")
                     for _ in range(CP)]

            # ---- phase 1: x load (2 HW-DGE queues) + GroupNorm stats ----
            # stats for tiles 0,2,3 via DVE bn_stats; tile 1 via ACT
            # Square/Identity with accum_out (free-dim sums) to halve the
            # serial DVE chain on the critical path.
            mv_sb = []
            with tc.tile_pool(name="xr", bufs=CP) as xr_pool:
                # [p, i, n] = x[ch = pair*256 + i*128 + p, n] in fp8
                x2_sb = [xr_pool.tile([P, 2, N], f8, tag="xr", name="xr")
                         for _ in range(CP)]
                xsl = [x2_sb[t // 2][:, t % 2, :] for t in range(CT)]
                st_sb = []
                acc_cols = []
                # tile 0 lands as 512-wide chunks so the first bn_stats can
                # start ~2us earlier; sync feeds DVE's tiles 0 then 2, scalar
                # feeds ACT's tile 1 first so its slower chain starts early,
                # then DVE's tile 3
                order = [(0, slice(ch * 512, (ch + 1) * 512), nc.sync)
                         for ch in range(4)]
                order += [(t, slice(ch * (N // 4), (ch + 1) * (N // 4)), eng)
                          for t, ch, eng in
                          [(1, 0, nc.scalar), (1, 1, nc.scalar),
                           (0, 2, nc.sync), (0, 3, nc.sync),
                           (1, 2, nc.scalar), (1, 3, nc.scalar),
                           (2, 0, nc.sync), (2, 1, nc.sync),
                           (3, 0, nc.scalar), (3, 1, nc.scalar),
                           (2, 2, nc.sync), (2, 3, nc.sync),
                           (3, 2, nc.scalar), (3, 3, nc.scalar)]]
                for t, csl, eng in order:
                    eng.dma_start(out=x2_sb[t // 2][:, t % 2, csl],
                                  in_=xr_d[t * P:(t + 1) * P, csl])

                for t in range(CT):
                    xt_g = xsl[t].rearrange("p (s f) -> p s f", f=512)
                    if t != 1:
                        st = stat_pool.tile([P, N // 512, 6], f32, tag="bnst",
                                            name="bnst")
                        sums = None
                        for s in range(N // 512):
                            nc.vector.bn_stats(out=st[:, s, :],
                                               in_=xt_g[:, s, :])
                    else:
                        st = None
                        sums = stat_pool.tile([P, 2, N // 512], f32, tag="acs",
                                              name="acs")
                        for s in range(N // 512):
                            scr = stat_pool.tile([P, 512], bf16, tag="scr",
                                                 name="scr", bufs=2)
                            nc.scalar.activation(
                                out=scr, in_=xt_g[:, s, :],
                                func=mybir.ActivationFunctionType.Square,
                                accum_out=sums[:, 1, s:s + 1])
                            nc.scalar.activation(
                                out=scr, in_=xt_g[:, s, :], func=ID,
                                accum_out=sums[:, 0, s:s + 1])
                    st_sb.append(st)
                    acc_cols.append(sums)

                # batched consts + weights + bv now (queues free after x)
                nc.sync.dma_start(out=inda_t, in_=inda_d[:, :])
                nc.sync.dma_start(out=indb_t, in_=indb_d[:, :])
                nc.sync.dma_start(out=ident_sb, in_=id_d[:, :])
                nc.sync.dma_start(out=ej_sb, in_=ej_d[:, :])
                for t in range(CT):
                    nc.sync.dma_start(out=cols_t[t],
                                      in_=cols_d[t * P:(t + 1) * P, :])
                worig_cm = tc.tile_pool(name="worig", bufs=1)
                worig_pool = worig_cm.__enter__()
                w_sb = {}
                for name in ("wkT", "wqT", "wvT"):
                    tiles = []
                    for t in range(CT):
                        tw = worig_pool.tile([P, C], bf16, tag=f"{name}{t}",
                                             name=f"{name}{t}")
                        nc.sync.dma_start(out=tw,
                                          in_=w_d[name][t * P:(t + 1) * P, :])
                        tiles.append(tw)
                    w_sb[name] = tiles
                # O-projection weights, fp8 DoubleRow layout (host-cast)
                wo2 = [const.tile([P, 2, C], f8, tag=f"wo2{pr}",
                                  name=f"wo2{pr}") for pr in range(CP)]
                for pr in range(CP):
                    for i in range(2):
                        nc.sync.dma_start(
                            out=wo2[pr][:, i, :],
                            in_=wo8_d[(2 * pr + i) * P:(2 * pr + i + 1) * P, :])
                bv_row = const.tile([1, C], f32, tag="bv_row", name="bv_row")
                nc.sync.dma_start(
                    out=bv_row,
                    in_=cols_d[:, 2:3].rearrange("c one -> one c"))

                for t in range(CT):
                    mv = stat_pool.tile([P, 2], f32, tag="mv", name="mv")
                    if st_sb[t] is not None:
                        nc.vector.bn_aggr(out=mv, in_=st_sb[t])
                        # mv = [mean, var] -> [mean, E[x^2]]
                        msq = stat_pool.tile([P, 1], f32, tag="msq", name="msq")
                        nc.vector.tensor_mul(msq, mv[:, 0:1], mv[:, 0:1])
                        nc.vector.tensor_add(mv[:, 1:2], mv[:, 1:2], msq)
                    else:
                        # sums[:, s, 0]=sum(x), [:, s, 1]=sum(x^2) per 512-chunk
                        sred = stat_pool.tile([P, 2], f32, tag="sred", name="sred")
                        nc.vector.tensor_reduce(
                            out=sred, in_=acc_cols[t],
                            op=mybir.AluOpType.add, axis=mybir.AxisListType.X)
                        nc.vector.tensor_scalar_mul(mv, sred, 1.0 / N)
                    mvb = stat_pool.tile([P, 2], bf16, tag="mvb", name="mvb")
                    nc.vector.tensor_copy(out=mvb, in_=mv)
                    mv_sb.append(mvb)

                # aggregate over channel groups: [32, 2] = [mean_g, E[x^2]_g]
                g_ps = pp_sm.tile([GROUPS, 2], f32, tag="den", name="den")
                for t in range(CT):
                    nc.tensor.matmul(g_ps, lhsT=inda_sb[t], rhs=mv_sb[t],
                                     start=(t == 0), stop=(t == CT - 1))
                g_sb = stat_pool.tile([GROUPS, 2], f32, tag="gsb", name="gsb")
                nc.vector.tensor_copy(out=g_sb, in_=g_ps)
                gm2 = stat_pool.tile([GROUPS, 1], f32, tag="gm2", name="gm2")
                nc.vector.tensor_mul(gm2, g_sb[:, 0:1], g_sb[:, 0:1])
                gvar = stat_pool.tile([GROUPS, 1], f32, tag="gvar", name="gvar")
                nc.vector.tensor_sub(gvar, g_sb[:, 1:2], gm2)
                eps_col = stat_pool.tile([GROUPS, 1], f32, tag="eps", name="eps")
                nc.vector.memset(eps_col, EPS)
                gstd = stat_pool.tile([GROUPS, 1], f32, tag="gstd", name="gstd")
                nc.scalar.activation(out=gstd, in_=gvar, func=SQRT, bias=eps_col)
                ga = stat_pool.tile([GROUPS, 1], f32, tag="ga", name="ga")
                nc.vector.reciprocal(out=ga, in_=gstd)
                coeffs = stat_pool.tile([GROUPS, 2], bf16, tag="coef", name="coef")
                nc.vector.tensor_copy(out=coeffs[:, 0:1], in_=ga)
                nc.vector.tensor_copy(out=coeffs[:, 1:2], in_=g_sb[:, 0:1])

                # broadcast group coeffs to per-channel scale/shift columns
                sc_cols = []
                tc_cols = []
                for t in range(CT):
                    b_ps = pp_sm.tile([P, 2], f32, tag="den", name="den")
                    nc.tensor.matmul(b_ps, lhsT=indb_sb[t], rhs=coeffs,
                                     start=True, stop=True)
                    bc = stat_pool.tile([P, 2], f32, tag="bc", name="bc")
                    nc.vector.tensor_copy(out=bc, in_=b_ps)
                    s_col = stat_pool.tile([P, 1], f32, tag="scol", name="scol")
                    nc.vector.tensor_mul(s_col, col_sb["gamma"][t], bc[:, 0:1])
                    tmp = stat_pool.tile([P, 1], f32, tag="tmp", name="tmp")
                    nc.vector.tensor_mul(tmp, bc[:, 1:2], s_col)
                    t_col = stat_pool.tile([P, 1], f32, tag="tcol", name="tcol")
                    nc.vector.tensor_sub(t_col, col_sb["beta"][t], tmp)
                    sc_cols.append(s_col)
                    tc_cols.append(t_col)

                # GroupNorm folding: wk@(s*x+t) = (wk*s)@x + wk@t.  Scale the
                # QKV weights per input channel into fp8 DoubleRow layout
                # [128, 2, C]; the wk@t bias corrections are tiny PE matmuls.
                tcb = []
                for t in range(CT):
                    tb = stat_pool.tile([P, 1], bf16, tag="tcb", name="tcb")
                    nc.vector.tensor_copy(out=tb, in_=tc_cols[t])
                    tcb.append(tb)
                w2 = {}
                for name in ("wkT", "wvT", "wqT"):
                    tiles = [const.tile([P, 2, C], f8, tag=f"{name}s{pr}",
                                        name=f"{name}s{pr}")
                             for pr in range(CP)]
                    for ci in range(CT):
                        dst = tiles[ci // 2][:, ci % 2, :]
                        if ci % 2 == 0:
                            nc.vector.tensor_scalar_mul(dst, w_sb[name][ci],
                                                        sc_cols[ci])
                        else:
                            nc.scalar.activation(out=dst, in_=w_sb[name][ci],
                                                 func=ID, scale=sc_cols[ci])
                    w2[name] = tiles

                # bias corrections: bq2[m] = bq[m] + sum_c wq[d,c] t_c
                # (K needs no bias at all: q_i.(k_j + c) with c constant over
                # j is softmax-invariant, so bk and the wk@t correction drop.)
                bias2 = {}
                for name, bcol in (("wqT", "bq"),):
                    cols2 = []
                    for m in range(CT):
                        tk_ps = pp_sm.tile([P, 1], f32, tag="den", name="den")
                        for ci in range(CT):
                            nc.tensor.matmul(
                                tk_ps,
                                lhsT=w_sb[name][ci][:, m * P:(m + 1) * P],
                                rhs=tcb[ci],
                                start=(ci == 0), stop=(ci == CT - 1))
                        b2 = stat_pool.tile([P, 1], f32, tag=f"b2{name}{m}",
                                            name=f"b2{name}{m}")
                        nc.vector.tensor_scalar(
                            out=b2, in0=tk_ps, scalar1=col_sb[bcol][m],
                            scalar2=None, op0=mybir.AluOpType.add)
                        cols2.append(b2)
                    bias2[name] = cols2
                # v bias row: bvt[c] = bv[c] + sum_c' t_c' wv[c,c'], broadcast
                tv_ps = pp_sm.tile([1, C], f32, tag="den", name="den")
                for ci in range(CT):
                    nc.tensor.matmul(tv_ps, lhsT=tcb[ci], rhs=w_sb["wvT"][ci],
                                     start=(ci == 0), stop=(ci == CT - 1))
                bvt_row = stat_pool.tile([1, C], f32, tag="bvtr", name="bvtr")
                nc.vector.tensor_add(bvt_row, tv_ps, bv_row)
                bvt_ps = pp_av.tile([P, IBS], f32, tag="pav", name="bvtps")
                nc.tensor.matmul(bvt_ps, lhsT=ones_rowf, rhs=bvt_row,
                                 start=True, stop=True)
                bvt_bcast = const.tile([P, C], f32, tag="bvt_bcast",
                                       name="bvt_bcast")
                nc.scalar.activation(out=bvt_bcast, in_=bvt_ps, func=ID)
                worig_cm.__exit__(None, None, None)

                # ---- phase 2: projections straight from fp8 x (DoubleRow) ----
                # K first (bias-free drains -> nothing to wait for), then V
                # (bvt ready by then), then Q (needs the bq2 bias columns).
                for nch in range(NCH):
                    hsl = slice(nch * IBS, (nch + 1) * IBS)
                    for m in range(CT):
                        ps = pp_mm.tile([P, IBS], f32, tag="mm", name="mm")
                        for pr in range(CP):
                            nc.tensor.matmul(
                                ps,
                                lhsT=w2["wkT"][pr][:, :, m * P:(m + 1) * P],
                                rhs=x2_sb[pr][:, :, hsl],
                                start=(pr == 0), stop=(pr == CP - 1),
                                perf_mode=DR)
                        nc.scalar.activation(
                            out=k2_sb[m // 2][:, m % 2, hsl], in_=ps, func=ID)

                # V^T projection; bias-add on DVE drains each PSUM right away
                # v2[jp][p, i, c] = V[c, key (2*jp+i)*128+p]
                v2_sb = []
                for jp in range(JP):
                    v2 = v_pool.tile([P, 2, C], f8, tag="v", name="v")
                    for i in range(2):
                        jt = 2 * jp + i
                        ps = pp_mm.tile([P, C], f32, tag="mm", name="mm")
                        for pr in range(CP):
                            nc.tensor.matmul(
                                ps,
                                lhsT=x2_sb[pr][:, :, jt * P:(jt + 1) * P],
                                rhs=w2["wvT"][pr],
                                start=(pr == 0), stop=(pr == CP - 1),
                                perf_mode=DR)
                        nc.vector.tensor_add(v2[:, i, :], ps, bvt_bcast)
                    v2_sb.append(v2)

                for nch in range(IB):
                    hsl = slice(nch * IBS, (nch + 1) * IBS)
                    for m in range(CT):
                        ps = pp_mm.tile([P, IBS], f32, tag="mm", name="mm")
                        for pr in range(CP):
                            nc.tensor.matmul(
                                ps,
                                lhsT=w2["wqT"][pr][:, :, m * P:(m + 1) * P],
                                rhs=x2_sb[pr][:, :, hsl],
                                start=(pr == 0), stop=(pr == CP - 1),
                                perf_mode=DR)
                        nc.scalar.activation(
                            out=q2_sb[m // 2][:, m % 2, hsl], in_=ps,
                            func=ID, bias=bias2["wqT"][m], scale=1.0)

            # ---- phase 3: attention + output proj + residual ----
            p_pool = ctx.enter_context(tc.tile_pool(name="p", bufs=10))
            xqb_pool = ctx.enter_context(tc.tile_pool(name="xqb", bufs=8))
            a_pool = ctx.enter_context(tc.tile_pool(name="a", bufs=2 * CT))
            o_pool = ctx.enter_context(tc.tile_pool(name="o", bufs=3))
            sm_pool = ctx.enter_context(tc.tile_pool(name="sm", bufs=2))

            LOOKAHEAD = 6  # key-tile pairs

            def emit_scores(bi, jp):
                """exp(score) tile for key pair jp: [p, i, query] fp8."""
                q0, ibs = bi * IBS, IBS
                isl = slice(q0, q0 + ibs)
                p2 = p_pool.tile([P, 2, ibs], f8, tag="p", name="p")
                for i in range(2):
                    jt = 2 * jp + i
                    ps = pp_mm.tile([P, ibs], f32, tag="mm", name="mm")
                    for pr in range(CP):
                        nc.tensor.matmul(
                            ps,
                            lhsT=k2_sb[pr][:, :, jt * P:(jt + 1) * P],
                            rhs=q2_sb[pr][:, :, isl],
                            start=(pr == 0), stop=(pr == CP - 1),
                            perf_mode=DR)
                    nc.scalar.activation(out=p2[:, i, :], in_=ps, func=EXP,
                                         scale=SM_SCALE, bias=ebias)
                return p2

            pending = {}
            NBLK = IB
            for bi in range(NBLK):
                q0, ibs = bi * IBS, IBS
                isl = slice(q0, q0 + ibs)
                nj = ibs // P
                # prefetch the residual tiles early so the o2 adds never wait
                xqb_l = []
                for dt_ in range(CT):
                    xqb_t = xqb_pool.tile([P, ibs], f32, tag="xqb", name="xqb")
                    nc.sync.dma_start(out=xqb_t,
                                      in_=xqb_d[dt_ * P:(dt_ + 1) * P, isl])
                    xqb_l.append(xqb_t)
                pav = [pp_av.tile([P, ibs], f32, tag="pav", name="pav")
                       for _ in range(CT)]
                acc = sm_pool.tile([P, ibs], f32, tag="acc", name="acc")
                accg = sm_pool.tile([P, ibs], f32, tag="accg", name="accg")
                for jp in range(JP):
                    p2 = pending.pop((bi, jp), None)
                    if p2 is None:
                        p2 = emit_scores(bi, jp)
                    # softmax denominator partials, split DVE/GPSIMD
                    if jp == 0:
                        nc.vector.tensor_copy(out=acc, in_=p2[:, 0, :])
                        nc.gpsimd.tensor_copy(out=accg, in_=p2[:, 1, :])
                    else:
                        nc.vector.tensor_add(acc, acc, p2[:, 0, :])
                        nc.gpsimd.tensor_add(accg, accg, p2[:, 1, :])
                    for m in range(CT):
                        nc.tensor.matmul(
                            pav[m],
                            lhsT=v2_sb[jp][:, :, m * P:(m + 1) * P],
                            rhs=p2,
                            start=(jp == 0), stop=(jp == JP - 1),
                            perf_mode=DR)

                nc.vector.tensor_add(acc, acc, accg)

                # denT[qp, j] = sum_p acc[p, j*128+qp]: transposed partition
                # reduce so the DVE reciprocal runs 128-wide (a [1, 512] row
                # would serialize on one DVE lane: ~4us -> ~0.1us).
                denT = pp_sm.tile([P, nj], f32, tag="den", name="den")
                for j in range(nj):
                    nc.tensor.matmul(denT[:, j:j + 1],
                                     lhsT=acc[:, j * P:(j + 1) * P],
                                     rhs=ones_colf, start=True, stop=True)
                recipT = sm_pool.tile([P, nj], f32, tag="recipT",
                                      name="recipT")
                nc.vector.reciprocal(out=recipT, in_=denT)
                tr_ps = pp_sm.tile([nj, P], f32, tag="den", name="den")
                nc.tensor.transpose(tr_ps, recipT, ident_sb)
                recip4 = sm_pool.tile([nj, P], f32, tag="recip4",
                                      name="recip4")
                nc.scalar.activation(out=recip4, in_=tr_ps, func=ID)
                # broadcast across partitions: bc[p, j*128+q] = recip4[j, q]
                # (lives in pp_sm: a pp_av slot would alias pav[0], whose
                # drain now depends on recip_b <- bc_ps -- a WAR cycle)
                bc_ps = pp_sm.tile([P, ibs], f32, tag="den", name="bcps")
                for j in range(nj):
                    nc.tensor.matmul(bc_ps[:, j * P:(j + 1) * P],
                                     lhsT=ej_sb[0:nj, j * P:(j + 1) * P],
                                     rhs=recip4, start=True, stop=True)
                recip_b = sm_pool.tile([P, ibs], f32, tag="recip_b",
                                       name="recip_b")
                nc.scalar.activation(out=recip_b, in_=bc_ps, func=ID)

                # score lookahead into the next block keeps the PE busy while
                # the normalize/O-proj tail of this block resolves; emitted
                # after the recip chain so its exps don't delay recip_b on ACT
                if bi + 1 < NBLK:
                    for la in range(LOOKAHEAD):
                        pending[(bi + 1, la)] = emit_scores(bi + 1, la)

                # normalized attention output straight to fp8 DoubleRow pairs
                # (the 1/den scale applied on the PSUM drain; DVE only -- it
                # reads PSUM)
                a8 = [a_pool.tile([P, 2, ibs], f8, tag="a", name="a")
                      for _ in range(CP)]
                for m in range(CT):
                    nc.vector.tensor_mul(a8[m // 2][:, m % 2, :], pav[m],
                                         recip_b)

                # O-projection (fp8 DoubleRow) + residual
                for dt_ in range(CT):
                    po = pp_mm.tile([P, ibs], f32, tag="mm", name="mm")
                    for pr in range(CP):
                        nc.tensor.matmul(
                            po,
                            lhsT=wo2[pr][:, :, dt_ * P:(dt_ + 1) * P],
                            rhs=a8[pr],
                            start=(pr == 0), stop=(pr == CP - 1),
                            perf_mode=DR)
                    o2 = o_pool.tile([P, ibs], f32, tag="o2", name="o2")
                    nc.vector.tensor_add(o2, po, xqb_l[dt_])
                    # spread the output DMAs over queues so the last block's
                    # stores drain in parallel
                    oeng = (nc.sync, nc.scalar, nc.gpsimd, nc.sync)[dt_]
                    oeng.dma_start(out=out_d[dt_ * P:(dt_ + 1) * P, isl],
                                   in_=o2)

    nc.finalize()
    return nc


def _make_consts():
    """Constant (core-independent) input arrays (packed)."""
    ind_a = np.zeros((P, CT * GROUPS), ml_dtypes.bfloat16)
    ind_b = np.zeros((GROUPS, CT * P), ml_dtypes.bfloat16)
    for t in range(CT):
        for p in range(P):
            g = (t * P + p) // GSIZE
            ind_a[p, t * GROUPS + g] = 1.0 / GSIZE
            ind_b[g, t * P + p] = 1.0
    return ind_a, ind_b


def _make_ej():
    ej = np.zeros((4, 4 * P), np.float32)
    for j in range(4):
        ej[j, j * P:(j + 1) * P] = 1.0
    return ej


def make_in_maps(x, gn_gamma, gn_beta, wq, bq, wk, bk, wv, bv, wo, bo):
    ind_a, ind_b = _make_consts()
    bf = ml_dtypes.bfloat16
    f8 = ml_dtypes.float8_e4m3
    cols = np.stack([np.asarray(a, np.float32) for a in
                     (bq, bk, bv, bo, gn_gamma, gn_beta)], axis=1)
    common = {
        "wqT": np.ascontiguousarray(np.asarray(wq, np.float32).T).astype(bf),
        "wkT": np.ascontiguousarray(np.asarray(wk, np.float32).T).astype(bf),
        "wvT": np.ascontiguousarray(np.asarray(wv, np.float32).T).astype(bf),
        "woT8": np.ascontiguousarray(np.asarray(wo, np.float32).T).astype(f8),
        "cols": np.ascontiguousarray(cols),
        "ind_a": ind_a,
        "ind_b": ind_b,
        "ident": np.eye(P, dtype=np.float32),
        "ej": _make_ej(),
    }
    x = np.asarray(x, np.float32)
    in_maps = []
    for core in range(N_CORES):
        b, half = divmod(core, 2)
        xb = x[b].reshape(C, N)
        xr = np.concatenate(
            [xb[:, half * NQ:(half + 1) * NQ],
             xb[:, (1 - half) * NQ:(2 - half) * NQ]],
            axis=1)
        xqb = xr[:, :NQ] + np.asarray(bo, np.float32).reshape(C, 1)
        in_maps.append({"xr": np.ascontiguousarray(xr).astype(f8),
                        "xqb": np.ascontiguousarray(xqb), **common})
    return in_maps


def gather_out(results):
    out = np.empty((B, C, N), np.float32)
    for core in range(N_CORES):
        b, half = divmod(core, 2)
        out[b][:, half * NQ:(half + 1) * NQ] = results[core]["out"]
    return out.reshape(B, C, H, W)


def get_nc():
    if "nc" not in _cache:
        _cache["nc"] = _build_nc()
    return _cache["nc"]


def kernel(**inputs):
    from concourse.bass_utils import run_bass_kernel_spmd

    nc = get_nc()
    in_maps = make_in_maps(**inputs)
    res = run_bass_kernel_spmd(nc, in_maps, list(range(N_CORES)))
    return gather_out(res.results)


if __name__ == "__main__":
    nc = _build_nc()
    print("built ok:", len(nc.m.functions[0].allocations), "allocations")
